# revision 24
# baseline (speedup 1.0000x reference)
"""Trainium2 Bass kernel for nn_DiscriminativeLoss.

Strategy: data-parallel over batch B=8 across the 8 NeuronCores (one sample
per core).  On each core:

  pass 1: segment sums + counts via one-hot matmuls (pixels on partitions,
          pixel = 2048*P + g layout loaded directly with a strided DMA).
  pass 2: per-pixel ||x - mu_{m}||^2 via a mean-gather matmul (4-way
          diagonal tile_position packing), DVE diff, ACT square, block-ones
          PE partition-reduction into a persistent PSUM d2 accumulator;
          then one dense sqrt + hinge^2 over the whole sample.
  pass 3: per-instance sums of hinge^2 via one-hot matmuls (h as stationary).

The three per-sample reductions (33x33 stats, 4x132 hinge sums) go back to
the host, which finishes the tiny per-sample/batch math in numpy.
"""

import sys

if "/opt/trn_rl_repo" not in sys.path:
    sys.path.insert(0, "/opt/trn_rl_repo")

from contextlib import ExitStack

import numpy as np

import concourse.bacc as bacc
import concourse.bass as bass
import concourse.mybir as mybir
import concourse.tile as tile
from concourse.bass_utils import run_bass_kernel_spmd

B, E, H, W = 8, 32, 512, 512
N_PIX = H * W  # 262144 per sample
NI = 32
C = NI + 1  # 33 segments (0 = background)
DELTA_V, DELTA_D = 0.5, 1.5
ALPHA, BETA, GAMMA = 1.0, 1.0, 0.001

F32 = mybir.dt.float32
BF16 = mybir.dt.bfloat16
I32 = mybir.dt.int32
AOP = mybir.AluOpType
ACT = mybir.ActivationFunctionType


def _bc(ap, extra_dims):
    """Append broadcast (step-0) free dims to an AP."""
    return bass.AP(tensor=ap.tensor, offset=ap.offset, ap=ap.ap + extra_dims)


def _squeeze(ap):
    """Drop count-1 free dims from an AP (keeps the partition dim)."""
    dims = [ap.ap[0]] + [d for d in ap.ap[1:] if d[1] > 1]
    return bass.AP(tensor=ap.tensor, offset=ap.offset, ap=dims)


def _ins(ap, mid_dims):
    """AP with broadcast (step-0) dims inserted between partition and free."""
    return bass.AP(
        tensor=ap.tensor,
        offset=ap.offset,
        ap=[ap.ap[0]] + mid_dims + list(ap.ap[1:]),
    )


def build_body(tc, nc, emb, mask, out_s, out_h, npix):
    """Emit the per-core program.  emb: (E, npix) f32, mask: (npix,) i32,
    out_s: (C, C) f32 [segment sums | counts], out_h: (4, 4*C) f32."""
    ctx = ExitStack()
    cols = npix // 128
    mask_r = mask.rearrange("(p g) -> p g", p=128)

    def emb_strided(g0, ng):
        """(128, E, ng) view of emb: [p, e, g] = emb[e, p*cols + g0 + g]."""
        sl = emb[:, g0 : g0 + ng]
        return bass.AP(
            tensor=sl.tensor,
            offset=sl.offset,
            ap=[[cols, 128], [npix, E], [1, ng]],
        )

    persist = ctx.enter_context(tc.tile_pool(name="persist", bufs=1))

    # ---- constants & mask staging -------------------------------------
    m_i32 = persist.tile([128, cols], I32)
    nc.sync.dma_start(out=m_i32[:], in_=mask_r)
    m_f32 = persist.tile([128, cols], F32)
    nc.vector.tensor_copy(out=m_f32[:], in_=m_i32[:])
    m_bf = persist.tile([128, cols], BF16)
    nc.vector.tensor_copy(out=m_bf[:], in_=m_f32[:])
    dram_pool = ctx.enter_context(tc.tile_pool(name="dscratch", bufs=1, space="DRAM"))
    m_dram = dram_pool.tile([128, cols], BF16)
    nc.sync.dma_start(out=m_dram[:], in_=m_bf[:])

    import ml_dtypes

    iota33_f = persist.tile([128, C], F32)
    nc.sync.dma_start(
        out=iota33_f[:],
        in_=nc.inline_tensor(
            np.broadcast_to(np.arange(C, dtype=np.float32), (128, C)).copy(),
            name="iota33",
        ).ap(),
    )

    # per-partition value (p % 32) + 1 broadcast along a row of width w_r
    w_r = min(1024, cols)
    iota_r = persist.tile([128, w_r], BF16)
    ir_np = ((np.arange(128) % 32) + 1).astype(ml_dtypes.bfloat16)
    nc.sync.dma_start(
        out=iota_r[:],
        in_=nc.inline_tensor(
            np.broadcast_to(ir_np[:, None], (128, w_r)).copy(), name="iotar"
        ).ap(),
    )

    blockones = persist.tile([128, 4], F32)
    bo_np = np.zeros((128, 4), np.float32)
    for q in range(4):
        bo_np[32 * q : 32 * q + 32, q] = 1.0
    nc.sync.dma_start(
        out=blockones[:], in_=nc.inline_tensor(bo_np, name="blockones").ap()
    )

    # pass-3 iota, c-major over groups of G3 columns (bf16)
    G3 = min(64, cols)
    i33g = persist.tile([128, C, G3], BF16)
    i33g_np = np.broadcast_to(
        np.arange(C, dtype=np.float32)[None, :, None], (128, C, G3)
    ).astype(ml_dtypes.bfloat16)
    nc.sync.dma_start(
        out=i33g[:], in_=nc.inline_tensor(i33g_np, name="i33g").ap()
    )

    b_eps = persist.tile([4, 1], F32)
    nc.vector.memset(b_eps[:], 1e-12)
    b_negdv = persist.tile([128, 1], F32)
    nc.vector.memset(b_negdv[:], -DELTA_V)

    dist_sb = persist.tile([128, cols], F32)
    h_bf = persist.tile([128, cols], BF16)
    S_sb = persist.tile([C, C], F32)
    means_all = persist.tile([C, E], F32)
    means_bf = persist.tile([C, E], BF16)
    means_fg4 = persist.tile([128, 32], BF16)
    Hs = persist.tile([4, 4 * C], F32)

    # ---- pass 1: segment sums + counts --------------------------------
    G1 = min(128, cols)
    n1 = cols // G1
    total1 = cols
    idx = 0
    with tc.tile_pool(name="p1", bufs=2) as p1p, tc.tile_pool(
        name="p1ps", bufs=1, space="PSUM"
    ) as pp1:
        psum_s = pp1.tile([C, C], F32)
        for t in range(n1):
            xt = p1p.tile([128, C, G1], F32, tag="xt")
            nc.sync.dma_start(
                out=_squeeze(xt[:, 0:E, :]), in_=emb_strided(t * G1, G1)
            )
            nc.gpsimd.memset(xt[:, E : E + 1, :], 1.0)
            o1 = p1p.tile([128, C, G1], F32, tag="o1")
            m_sl = m_f32[:, t * G1 : (t + 1) * G1]
            nc.vector.tensor_tensor(
                out=o1[:],
                in0=_ins(m_sl, [[0, C]]),
                in1=_bc(iota33_f[:], [[0, G1]]),
                op=AOP.is_equal,
            )
            for g in range(G1):
                nc.tensor.matmul(
                    psum_s[:],
                    lhsT=_squeeze(o1[:, :, g : g + 1]),
                    rhs=_squeeze(xt[:, :, g : g + 1]),
                    start=(idx == 0),
                    stop=(idx == total1 - 1),
                )
                idx += 1
        nc.vector.tensor_copy(out=S_sb[:], in_=psum_s[:])
    nc.sync.dma_start(out=out_s, in_=S_sb[:])

    # ---- means --------------------------------------------------------
    cnt_m = persist.tile([C, 1], F32)
    nc.vector.tensor_scalar_max(cnt_m[:], S_sb[:, E : E + 1], 1.0)
    recip = persist.tile([C, 1], F32)
    nc.vector.reciprocal(out=recip[:], in_=cnt_m[:])
    nc.vector.tensor_scalar_mul(means_all[:], S_sb[:, 0:E], recip[:, 0:1])
    nc.vector.tensor_copy(out=means_bf[:], in_=means_all[:])
    # foreground means (instance r+1 -> row r), replicated to all 4 quadrants
    means_dram = dram_pool.tile([32, 32], BF16)
    nc.sync.dma_start(out=means_dram[:], in_=means_bf[1:C, :])
    for q in range(4):
        nc.sync.dma_start(
            out=means_fg4[32 * q : 32 * q + 32, :], in_=means_dram[:]
        )

    # ---- pass 2: per-pixel distances ----------------------------------
    hw2 = cols // 2
    nv = (hw2 + 511) // 512
    with tc.tile_pool(name="p2", bufs=2) as p2p, tc.tile_pool(
        name="p2mu", bufs=2, space="PSUM"
    ) as pmu, tc.tile_pool(name="p2d2", bufs=2, space="PSUM") as pd2:
        for s in range(32):
            x8 = p2p.tile([128, cols], F32, tag="x8")
            mbc = p2p.tile([128, cols], BF16, tag="mbc")
            for q in range(4):
                row = 4 * s + q
                nc.sync.dma_start(
                    out=x8[32 * q : 32 * q + 32, :],
                    in_=emb[:, row * cols : (row + 1) * cols],
                )
                src = m_dram[row : row + 1, :]
                nc.gpsimd.dma_start(
                    out=mbc[32 * q : 32 * q + 32, :],
                    in_=bass.AP(
                        tensor=src.tensor,
                        offset=src.offset,
                        ap=[[0, 32]] + src.ap[1:],
                    ),
                )
            o2 = p2p.tile([128, cols], BF16, tag="o2")
            nc.vector.tensor_tensor(
                out=o2[:],
                in0=mbc[:],
                in1=_ins(iota_r[:], [[0, cols // w_r]]),
                op=AOP.is_equal,
            )
            for h2 in range(2):
                muh = pmu.tile([128, hw2], F32, tag="mu")
                c0 = h2 * hw2
                for q in range(4):
                    for v in range(nv):
                        v0, v1 = v * 512, min((v + 1) * 512, hw2)
                        nc.tensor.matmul(
                            muh[32 * q : 32 * q + 32, v0:v1],
                            lhsT=means_fg4[32 * q : 32 * q + 32, :],
                            rhs=o2[32 * q : 32 * q + 32, c0 + v0 : c0 + v1],
                            start=True,
                            stop=True,
                            tile_position=(32 * q, 32 * q),
                        )
                diffh = p2p.tile([128, hw2], F32, tag="diff")
                nc.vector.tensor_tensor(
                    out=diffh[:], in0=x8[:, c0 : c0 + hw2], in1=muh[:], op=AOP.subtract
                )
                sqh = p2p.tile([128, hw2], F32, tag="sq")
                nc.scalar.activation(out=sqh[:], in_=diffh[:], func=ACT.Square)
                d2t = pd2.tile([4, hw2], F32, tag="d2")
                for v in range(nv):
                    v0, v1 = v * 512, min((v + 1) * 512, hw2)
                    nc.tensor.matmul(
                        d2t[:, v0:v1],
                        lhsT=blockones[:],
                        rhs=sqh[:, v0:v1],
                        start=True,
                        stop=True,
                    )
                dist_t = p2p.tile([4, hw2], F32, tag="dst")
                nc.scalar.activation(
                    out=dist_t[:], in_=d2t[:], func=ACT.Sqrt, bias=b_eps[:]
                )
                nc.sync.dma_start(
                    out=dist_sb[4 * s : 4 * s + 4, c0 : c0 + hw2], in_=dist_t[:]
                )
    r_t = persist.tile([128, cols], F32)
    nc.scalar.activation(
        out=r_t[:], in_=dist_sb[:], func=ACT.Relu, bias=b_negdv[:]
    )
    nc.vector.tensor_tensor(out=h_bf[:], in0=r_t[:], in1=r_t[:], op=AOP.mult)

    # ---- pass 3: per-instance hinge sums ------------------------------
    n3 = cols // G3
    nq = G3 // 4
    total3 = n3 * nq
    idx = 0
    with tc.tile_pool(name="p3", bufs=2) as p3p, tc.tile_pool(
        name="p3ps", bufs=1, space="PSUM"
    ) as pp3:
        psum_h = pp3.tile([4, 4 * C], F32)
        for t in range(n3):
            o3 = p3p.tile([128, C, G3], BF16, tag="o3")
            m_sl = m_bf[:, t * G3 : (t + 1) * G3]
            nc.vector.tensor_tensor(
                out=o3[:],
                in0=_ins(m_sl, [[0, C]]),
                in1=i33g[:],
                op=AOP.is_equal,
            )
            for i in range(nq):
                g0 = t * G3 + 4 * i
                nc.tensor.matmul(
                    psum_h[:],
                    lhsT=h_bf[:, g0 : g0 + 4],
                    rhs=o3[:, :, 4 * i : 4 * i + 4].rearrange("p c g -> p g c"),
                    start=(idx == 0),
                    stop=(idx == total3 - 1),
                )
                idx += 1
        nc.vector.tensor_copy(out=Hs[:], in_=psum_h[:])
    nc.sync.dma_start(out=out_h, in_=Hs[:])
    ctx.close()


def build_program(npix=N_PIX):
    nc = bacc.Bacc("TRN2", debug=False, num_devices=8)
    emb = nc.dram_tensor("emb", (E, npix), F32, kind="ExternalInput").ap()
    mask = nc.dram_tensor("mask", (npix,), I32, kind="ExternalInput").ap()
    out_s = nc.dram_tensor("out_s", (C, C), F32, kind="ExternalOutput").ap()
    out_h = nc.dram_tensor("out_h", (4, 4 * C), F32, kind="ExternalOutput").ap()
    with tile.TileContext(nc) as tc:
        build_body(tc, nc, emb, mask, out_s, out_h, npix)
    nc.compile()
    return nc


_program = None


def _get_program():
    global _program
    if _program is None:
        _program = build_program()
    return _program


def finalize_host(S, Hdiag):
    """Per-sample host math.  S: (C, C) [sums | counts], Hdiag: (4, 4*C)."""
    S = S.astype(np.float64)
    Hh = Hdiag.astype(np.float64)
    Hc = np.zeros(C)
    for i in range(4):
        Hc += Hh[i, C * i : C * i + C]
    counts = S[:, E]
    sums = S[:, 0:E]
    means_all = sums / np.maximum(counts, 1.0)[:, None]
    means = means_all[1:]
    cnt = counts[1:]
    present = cnt > 0

    per_inst = Hc[1:] / np.maximum(cnt, 1.0)
    n_present = max(present.sum(), 1.0)
    var_loss = np.where(present, per_inst, 0.0).sum() / n_present

    d = means[:, None, :] - means[None, :, :]
    sq = (d * d).sum(-1)
    iu = np.triu(np.ones((NI, NI), bool), k=1)
    pd = np.sqrt(np.where(iu, np.maximum(sq, 1e-12), 1.0))
    hinged_d = np.where(iu, np.maximum(2.0 * DELTA_D - pd, 0.0) ** 2, 0.0)
    dist_loss = hinged_d.sum() / (NI * (NI - 1) / 2.0)

    reg_loss = np.sqrt((means * means).sum(1) + 1e-12).mean()
    return var_loss, dist_loss, reg_loss


_last_results = None


def kernel(embeddings, instance_masks, trace=False):
    global _last_results
    nc = _get_program()
    embeddings = np.ascontiguousarray(np.asarray(embeddings), dtype=np.float32)
    instance_masks = np.ascontiguousarray(
        np.asarray(instance_masks), dtype=np.int32
    )
    in_maps = [
        {
            "emb": embeddings[b].reshape(E, N_PIX),
            "mask": instance_masks[b].reshape(N_PIX),
        }
        for b in range(B)
    ]
    res = run_bass_kernel_spmd(nc, in_maps, core_ids=list(range(B)), trace=trace)
    _last_results = res
    var = np.zeros(B)
    dst = np.zeros(B)
    reg = np.zeros(B)
    for b in range(B):
        var[b], dst[b], reg[b] = finalize_host(
            res.results[b]["out_s"], res.results[b]["out_h"]
        )
    var_loss = var.mean()
    dist_loss = dst.mean()
    reg_loss = reg.mean()
    total = ALPHA * var_loss + BETA * dist_loss + GAMMA * reg_loss
    return (
        np.float32(total),
        np.float32(var_loss),
        np.float32(dist_loss),
        np.float32(reg_loss),
    )


# revision 25
# speedup vs baseline: 4445.6794x; 4445.6794x over previous
"""Trainium2 Bass kernel for nn_DiscriminativeLoss.

Strategy: data-parallel over batch B=8 across the 8 NeuronCores (one sample
per core).  On each core:

  pass 1: segment sums + counts via one-hot matmuls (pixels on partitions,
          pixel = 2048*P + g layout loaded directly with a strided DMA).
  pass 2: per-pixel ||x - mu_{m}||^2 via a mean-gather matmul (4-way
          diagonal tile_position packing), DVE diff, ACT square, block-ones
          PE partition-reduction into a persistent PSUM d2 accumulator;
          then one dense sqrt + hinge^2 over the whole sample.
  pass 3: per-instance sums of hinge^2 via one-hot matmuls (h as stationary).

The three per-sample reductions (33x33 stats, 4x132 hinge sums) go back to
the host, which finishes the tiny per-sample/batch math in numpy.
"""

import sys

if "/opt/trn_rl_repo" not in sys.path:
    sys.path.insert(0, "/opt/trn_rl_repo")

from contextlib import ExitStack

import numpy as np

import concourse.bacc as bacc
import concourse.bass as bass
import concourse.mybir as mybir
import concourse.tile as tile
from concourse.bass_utils import run_bass_kernel_spmd

B, E, H, W = 8, 32, 512, 512
N_PIX = H * W  # 262144 per sample
NI = 32
C = NI + 1  # 33 segments (0 = background)
DELTA_V, DELTA_D = 0.5, 1.5
ALPHA, BETA, GAMMA = 1.0, 1.0, 0.001

F32 = mybir.dt.float32
BF16 = mybir.dt.bfloat16
I32 = mybir.dt.int32
AOP = mybir.AluOpType
ACT = mybir.ActivationFunctionType


def _bc(ap, extra_dims):
    """Append broadcast (step-0) free dims to an AP."""
    return bass.AP(tensor=ap.tensor, offset=ap.offset, ap=ap.ap + extra_dims)


def _squeeze(ap):
    """Drop count-1 free dims from an AP (keeps the partition dim)."""
    dims = [ap.ap[0]] + [d for d in ap.ap[1:] if d[1] > 1]
    return bass.AP(tensor=ap.tensor, offset=ap.offset, ap=dims)


def _ins(ap, mid_dims):
    """AP with broadcast (step-0) dims inserted between partition and free."""
    return bass.AP(
        tensor=ap.tensor,
        offset=ap.offset,
        ap=[ap.ap[0]] + mid_dims + list(ap.ap[1:]),
    )


def build_body(tc, nc, emb, mask, out_s, out_h, npix):
    """Emit the per-core program.  emb: (E, npix) f32, mask: (npix,) i32,
    out_s: (C, C) f32 [segment sums | counts], out_h: (4, 4*C) f32."""
    ctx = ExitStack()
    cols = npix // 128
    mask_r = mask.rearrange("(p g) -> p g", p=128)

    def emb_strided(g0, ng):
        """(128, E, ng) view of emb: [p, e, g] = emb[e, p*cols + g0 + g]."""
        sl = emb[:, g0 : g0 + ng]
        return bass.AP(
            tensor=sl.tensor,
            offset=sl.offset,
            ap=[[cols, 128], [npix, E], [1, ng]],
        )

    persist = ctx.enter_context(tc.tile_pool(name="persist", bufs=1))

    # ---- constants & mask staging -------------------------------------
    m_i32 = persist.tile([128, cols], I32)
    nc.sync.dma_start(out=m_i32[:], in_=mask_r)
    m_f32 = persist.tile([128, cols], F32)
    nc.vector.tensor_copy(out=m_f32[:], in_=m_i32[:])
    m_bf = persist.tile([128, cols], BF16)
    nc.vector.tensor_copy(out=m_bf[:], in_=m_f32[:])
    dram_pool = ctx.enter_context(tc.tile_pool(name="dscratch", bufs=1, space="DRAM"))
    m_dram = dram_pool.tile([128, cols], BF16)
    nc.sync.dma_start(out=m_dram[:], in_=m_bf[:])

    import ml_dtypes

    iota33_f = persist.tile([128, C], F32)
    nc.sync.dma_start(
        out=iota33_f[:],
        in_=nc.inline_tensor(
            np.broadcast_to(np.arange(C, dtype=np.float32), (128, C)).copy(),
            name="iota33",
        ).ap(),
    )

    # per-partition value (p % 32) + 1 broadcast along a row of width w_r
    w_r = min(1024, cols)
    iota_r = persist.tile([128, w_r], BF16)
    ir_np = ((np.arange(128) % 32) + 1).astype(ml_dtypes.bfloat16)
    nc.sync.dma_start(
        out=iota_r[:],
        in_=nc.inline_tensor(
            np.broadcast_to(ir_np[:, None], (128, w_r)).copy(), name="iotar"
        ).ap(),
    )

    blockones = persist.tile([128, 4], F32)
    bo_np = np.zeros((128, 4), np.float32)
    for q in range(4):
        bo_np[32 * q : 32 * q + 32, q] = 1.0
    nc.sync.dma_start(
        out=blockones[:], in_=nc.inline_tensor(bo_np, name="blockones").ap()
    )

    # pass-3 iota, c-major over groups of G3 columns (bf16)
    G3 = min(64, cols)
    i33g = persist.tile([128, C, G3], BF16)
    i33g_np = np.broadcast_to(
        np.arange(C, dtype=np.float32)[None, :, None], (128, C, G3)
    ).astype(ml_dtypes.bfloat16)
    nc.sync.dma_start(
        out=i33g[:], in_=nc.inline_tensor(i33g_np, name="i33g").ap()
    )

    b_eps = persist.tile([4, 1], F32)
    nc.vector.memset(b_eps[:], 1e-12)
    b_negdv = persist.tile([128, 1], F32)
    nc.vector.memset(b_negdv[:], -DELTA_V)

    dist_sb = persist.tile([128, cols], F32)
    h_bf = persist.tile([128, cols], BF16)
    S_sb = persist.tile([C, C], F32)
    means_all = persist.tile([C, E], F32)
    means_bf = persist.tile([C, E], BF16)
    means_fg4 = persist.tile([128, 32], BF16)
    Hs = persist.tile([4, 4 * C], F32)

    # ---- pass 1: segment sums + counts --------------------------------
    G1 = min(128, cols)
    n1 = cols // G1
    total1 = cols
    idx = 0
    with tc.tile_pool(name="p1", bufs=2) as p1p, tc.tile_pool(
        name="p1ps", bufs=1, space="PSUM"
    ) as pp1:
        psum_s = pp1.tile([C, C], F32)
        for t in range(n1):
            xt = p1p.tile([128, C, G1], F32, tag="xt")
            nc.sync.dma_start(
                out=_squeeze(xt[:, 0:E, :]), in_=emb_strided(t * G1, G1)
            )
            nc.gpsimd.memset(xt[:, E : E + 1, :], 1.0)
            o1 = p1p.tile([128, C, G1], F32, tag="o1")
            m_sl = m_f32[:, t * G1 : (t + 1) * G1]
            nc.vector.tensor_tensor(
                out=o1[:],
                in0=_ins(m_sl, [[0, C]]),
                in1=_bc(iota33_f[:], [[0, G1]]),
                op=AOP.is_equal,
            )
            for g in range(G1):
                nc.tensor.matmul(
                    psum_s[:],
                    lhsT=_squeeze(o1[:, :, g : g + 1]),
                    rhs=_squeeze(xt[:, :, g : g + 1]),
                    start=(idx == 0),
                    stop=(idx == total1 - 1),
                )
                idx += 1
        nc.vector.tensor_copy(out=S_sb[:], in_=psum_s[:])
    nc.sync.dma_start(out=out_s, in_=S_sb[:])

    # ---- means --------------------------------------------------------
    cnt_m = persist.tile([C, 1], F32)
    nc.vector.tensor_scalar_max(cnt_m[:], S_sb[:, E : E + 1], 1.0)
    recip = persist.tile([C, 1], F32)
    nc.vector.reciprocal(out=recip[:], in_=cnt_m[:])
    nc.vector.tensor_scalar_mul(means_all[:], S_sb[:, 0:E], recip[:, 0:1])
    nc.vector.tensor_copy(out=means_bf[:], in_=means_all[:])
    # foreground means (instance r+1 -> row r), replicated to all 4 quadrants
    means_dram = dram_pool.tile([32, 32], BF16)
    nc.sync.dma_start(out=means_dram[:], in_=means_bf[1:C, :])
    for q in range(4):
        nc.sync.dma_start(
            out=means_fg4[32 * q : 32 * q + 32, :], in_=means_dram[:]
        )

    # ---- pass 2: per-pixel distances ----------------------------------
    hw2 = cols // 2
    nv = (hw2 + 511) // 512
    with tc.tile_pool(name="p2", bufs=2) as p2p, tc.tile_pool(
        name="p2mu", bufs=2, space="PSUM"
    ) as pmu, tc.tile_pool(name="p2d2", bufs=2, space="PSUM") as pd2:
        for s in range(32):
            x8 = p2p.tile([128, cols], F32, tag="x8")
            mbc = p2p.tile([128, cols], BF16, tag="mbc")
            for q in range(4):
                row = 4 * s + q
                nc.sync.dma_start(
                    out=x8[32 * q : 32 * q + 32, :],
                    in_=emb[:, row * cols : (row + 1) * cols],
                )
                src = m_dram[row : row + 1, :]
                nc.gpsimd.dma_start(
                    out=mbc[32 * q : 32 * q + 32, :],
                    in_=bass.AP(
                        tensor=src.tensor,
                        offset=src.offset,
                        ap=[[0, 32]] + src.ap[1:],
                    ),
                )
            o2 = p2p.tile([128, cols], BF16, tag="o2")
            nc.vector.tensor_tensor(
                out=o2[:],
                in0=mbc[:],
                in1=_ins(iota_r[:], [[0, cols // w_r]]),
                op=AOP.is_equal,
            )
            for h2 in range(2):
                muh = pmu.tile([128, hw2], F32, tag="mu")
                c0 = h2 * hw2
                for q in range(4):
                    for v in range(nv):
                        v0, v1 = v * 512, min((v + 1) * 512, hw2)
                        nc.tensor.matmul(
                            muh[32 * q : 32 * q + 32, v0:v1],
                            lhsT=means_fg4[32 * q : 32 * q + 32, :],
                            rhs=o2[32 * q : 32 * q + 32, c0 + v0 : c0 + v1],
                            start=True,
                            stop=True,
                            tile_position=(32 * q, 32 * q),
                        )
                diffh = p2p.tile([128, hw2], F32, tag="diff")
                nc.vector.tensor_tensor(
                    out=diffh[:], in0=x8[:, c0 : c0 + hw2], in1=muh[:], op=AOP.subtract
                )
                sqh = p2p.tile([128, hw2], F32, tag="sq")
                nc.scalar.activation(out=sqh[:], in_=diffh[:], func=ACT.Square)
                d2t = pd2.tile([4, hw2], F32, tag="d2")
                for v in range(nv):
                    v0, v1 = v * 512, min((v + 1) * 512, hw2)
                    nc.tensor.matmul(
                        d2t[:, v0:v1],
                        lhsT=blockones[:],
                        rhs=sqh[:, v0:v1],
                        start=True,
                        stop=True,
                    )
                dist_t = p2p.tile([4, hw2], F32, tag="dst")
                nc.scalar.activation(
                    out=dist_t[:], in_=d2t[:], func=ACT.Sqrt, bias=b_eps[:]
                )
                nc.sync.dma_start(
                    out=dist_sb[4 * s : 4 * s + 4, c0 : c0 + hw2], in_=dist_t[:]
                )
    r_t = persist.tile([128, cols], F32)
    nc.scalar.activation(
        out=r_t[:], in_=dist_sb[:], func=ACT.Relu, bias=b_negdv[:]
    )
    nc.vector.tensor_tensor(out=h_bf[:], in0=r_t[:], in1=r_t[:], op=AOP.mult)

    # ---- pass 3: per-instance hinge sums ------------------------------
    n3 = cols // G3
    nq = G3 // 4
    total3 = n3 * nq
    idx = 0
    with tc.tile_pool(name="p3", bufs=2) as p3p, tc.tile_pool(
        name="p3ps", bufs=1, space="PSUM"
    ) as pp3:
        psum_h = pp3.tile([4, 4 * C], F32)
        for t in range(n3):
            o3 = p3p.tile([128, C, G3], BF16, tag="o3")
            m_sl = m_bf[:, t * G3 : (t + 1) * G3]
            nc.vector.tensor_tensor(
                out=o3[:],
                in0=_ins(m_sl, [[0, C]]),
                in1=i33g[:],
                op=AOP.is_equal,
            )
            for i in range(nq):
                g0 = t * G3 + 4 * i
                nc.tensor.matmul(
                    psum_h[:],
                    lhsT=h_bf[:, g0 : g0 + 4],
                    rhs=o3[:, :, 4 * i : 4 * i + 4].rearrange("p c g -> p g c"),
                    start=(idx == 0),
                    stop=(idx == total3 - 1),
                )
                idx += 1
        nc.vector.tensor_copy(out=Hs[:], in_=psum_h[:])
    nc.sync.dma_start(out=out_h, in_=Hs[:])
    ctx.close()


def build_program(npix=N_PIX, loop_k=None):
    nc = bacc.Bacc("TRN2", debug=False, num_devices=8)
    emb = nc.dram_tensor("emb", (E, npix), F32, kind="ExternalInput").ap()
    mask = nc.dram_tensor("mask", (npix,), I32, kind="ExternalInput").ap()
    out_s = nc.dram_tensor("out_s", (C, C), F32, kind="ExternalOutput").ap()
    out_h = nc.dram_tensor("out_h", (4, 4 * C), F32, kind="ExternalOutput").ap()
    with tile.TileContext(nc) as tc:
        if loop_k is None:
            build_body(tc, nc, emb, mask, out_s, out_h, npix)
        else:
            with tc.For_i(0, loop_k, 1):
                build_body(tc, nc, emb, mask, out_s, out_h, npix)
    nc.compile()
    return nc


_program = None


def _get_program():
    global _program
    if _program is None:
        _program = build_program()
    return _program


def finalize_host(S, Hdiag):
    """Per-sample host math.  S: (C, C) [sums | counts], Hdiag: (4, 4*C)."""
    S = S.astype(np.float64)
    Hh = Hdiag.astype(np.float64)
    Hc = np.zeros(C)
    for i in range(4):
        Hc += Hh[i, C * i : C * i + C]
    counts = S[:, E]
    sums = S[:, 0:E]
    means_all = sums / np.maximum(counts, 1.0)[:, None]
    means = means_all[1:]
    cnt = counts[1:]
    present = cnt > 0

    per_inst = Hc[1:] / np.maximum(cnt, 1.0)
    n_present = max(present.sum(), 1.0)
    var_loss = np.where(present, per_inst, 0.0).sum() / n_present

    d = means[:, None, :] - means[None, :, :]
    sq = (d * d).sum(-1)
    iu = np.triu(np.ones((NI, NI), bool), k=1)
    pd = np.sqrt(np.where(iu, np.maximum(sq, 1e-12), 1.0))
    hinged_d = np.where(iu, np.maximum(2.0 * DELTA_D - pd, 0.0) ** 2, 0.0)
    dist_loss = hinged_d.sum() / (NI * (NI - 1) / 2.0)

    reg_loss = np.sqrt((means * means).sum(1) + 1e-12).mean()
    return var_loss, dist_loss, reg_loss


_last_results = None


def kernel(embeddings, instance_masks, trace=False):
    global _last_results
    nc = _get_program()
    embeddings = np.ascontiguousarray(np.asarray(embeddings), dtype=np.float32)
    instance_masks = np.ascontiguousarray(
        np.asarray(instance_masks), dtype=np.int32
    )
    in_maps = [
        {
            "emb": embeddings[b].reshape(E, N_PIX),
            "mask": instance_masks[b].reshape(N_PIX),
        }
        for b in range(B)
    ]
    res = run_bass_kernel_spmd(nc, in_maps, core_ids=list(range(B)), trace=trace)
    _last_results = res
    var = np.zeros(B)
    dst = np.zeros(B)
    reg = np.zeros(B)
    for b in range(B):
        var[b], dst[b], reg[b] = finalize_host(
            res.results[b]["out_s"], res.results[b]["out_h"]
        )
    var_loss = var.mean()
    dist_loss = dst.mean()
    reg_loss = reg.mean()
    total = ALPHA * var_loss + BETA * dist_loss + GAMMA * reg_loss
    return (
        np.float32(total),
        np.float32(var_loss),
        np.float32(dist_loss),
        np.float32(reg_loss),
    )


# revision 27
# speedup vs baseline: 5211.2148x; 1.1722x over previous
"""Trainium2 Bass kernel for nn_DiscriminativeLoss.

Strategy: data-parallel over batch B=8 across the 8 NeuronCores (one sample
per core).  On each core:

  pass 1: segment sums + counts via one-hot matmuls (pixels on partitions,
          pixel = 2048*P + g layout loaded directly with a strided DMA).
  pass 2: per-pixel ||x - mu_{m}||^2 via a mean-gather matmul (4-way
          diagonal tile_position packing), DVE diff, ACT square, block-ones
          PE partition-reduction into a persistent PSUM d2 accumulator;
          then one dense sqrt + hinge^2 over the whole sample.
  pass 3: per-instance sums of hinge^2 via one-hot matmuls (h as stationary).

The three per-sample reductions (33x33 stats, 4x132 hinge sums) go back to
the host, which finishes the tiny per-sample/batch math in numpy.
"""

import sys

if "/opt/trn_rl_repo" not in sys.path:
    sys.path.insert(0, "/opt/trn_rl_repo")

from contextlib import ExitStack

import numpy as np

import concourse.bacc as bacc
import concourse.bass as bass
import concourse.mybir as mybir
import concourse.tile as tile
from concourse.bass_utils import run_bass_kernel_spmd

B, E, H, W = 8, 32, 512, 512
N_PIX = H * W  # 262144 per sample
NI = 32
C = NI + 1  # 33 segments (0 = background)
DELTA_V, DELTA_D = 0.5, 1.5
ALPHA, BETA, GAMMA = 1.0, 1.0, 0.001

F32 = mybir.dt.float32
BF16 = mybir.dt.bfloat16
I32 = mybir.dt.int32
AOP = mybir.AluOpType
ACT = mybir.ActivationFunctionType


def _bc(ap, extra_dims):
    """Append broadcast (step-0) free dims to an AP."""
    return bass.AP(tensor=ap.tensor, offset=ap.offset, ap=ap.ap + extra_dims)


def _squeeze(ap):
    """Drop count-1 free dims from an AP (keeps the partition dim)."""
    dims = [ap.ap[0]] + [d for d in ap.ap[1:] if d[1] > 1]
    return bass.AP(tensor=ap.tensor, offset=ap.offset, ap=dims)


def _ins(ap, mid_dims):
    """AP with broadcast (step-0) dims inserted between partition and free."""
    return bass.AP(
        tensor=ap.tensor,
        offset=ap.offset,
        ap=[ap.ap[0]] + mid_dims + list(ap.ap[1:]),
    )


def build_body(tc, nc, emb, mask, out_s, out_h, npix):
    """Emit the per-core program.  emb: (E, npix) f32, mask: (npix,) i32,
    out_s: (C, C) f32 [segment sums | counts], out_h: (4, 4*C) f32."""
    ctx = ExitStack()
    cols = npix // 128
    mask_r = mask.rearrange("(p g) -> p g", p=128)

    def emb_strided(g0, ng):
        """(128, E, ng) view of emb: [p, e, g] = emb[e, p*cols + g0 + g]."""
        sl = emb[:, g0 : g0 + ng]
        return bass.AP(
            tensor=sl.tensor,
            offset=sl.offset,
            ap=[[cols, 128], [npix, E], [1, ng]],
        )

    persist = ctx.enter_context(tc.tile_pool(name="persist", bufs=1))

    # ---- constants & mask staging -------------------------------------
    m_i32 = persist.tile([128, cols], I32)
    nc.sync.dma_start(out=m_i32[:], in_=mask_r)
    m_f32 = persist.tile([128, cols], F32)
    nc.vector.tensor_copy(out=m_f32[:], in_=m_i32[:])
    m_bf = persist.tile([128, cols], BF16)
    nc.vector.tensor_copy(out=m_bf[:], in_=m_f32[:])
    dram_pool = ctx.enter_context(tc.tile_pool(name="dscratch", bufs=1, space="DRAM"))
    m_dram = dram_pool.tile([128, cols], BF16)
    nc.sync.dma_start(out=m_dram[:], in_=m_bf[:])

    import ml_dtypes

    iota33_f = persist.tile([128, C], F32)
    nc.sync.dma_start(
        out=iota33_f[:],
        in_=nc.inline_tensor(
            np.broadcast_to(np.arange(C, dtype=np.float32), (128, C)).copy(),
            name="iota33",
        ).ap(),
    )

    # per-partition value (p % 32) + 1 broadcast along a row of width w_r
    w_r = min(1024, cols)
    iota_r = persist.tile([128, w_r], BF16)
    ir_np = ((np.arange(128) % 32) + 1).astype(ml_dtypes.bfloat16)
    nc.sync.dma_start(
        out=iota_r[:],
        in_=nc.inline_tensor(
            np.broadcast_to(ir_np[:, None], (128, w_r)).copy(), name="iotar"
        ).ap(),
    )

    blockones = persist.tile([128, 4], F32)
    bo_np = np.zeros((128, 4), np.float32)
    for q in range(4):
        bo_np[32 * q : 32 * q + 32, q] = 1.0
    nc.sync.dma_start(
        out=blockones[:], in_=nc.inline_tensor(bo_np, name="blockones").ap()
    )

    # one-hot compare iota, c-major over groups of G3 columns (bf16)
    G3 = min(64, cols)
    i33g = persist.tile([128, C, G3], BF16)
    i33g_np = np.broadcast_to(
        np.arange(C, dtype=np.float32)[None, :, None], (128, C, G3)
    ).astype(ml_dtypes.bfloat16)
    nc.sync.dma_start(
        out=i33g[:], in_=nc.inline_tensor(i33g_np, name="i33g").ap()
    )

    ident33 = persist.tile([C, C], F32)
    nc.sync.dma_start(
        out=ident33[:],
        in_=nc.inline_tensor(np.eye(C, dtype=np.float32), name="ident33").ap(),
    )

    b_eps = persist.tile([4, 1], F32)
    nc.vector.memset(b_eps[:], 1e-12)
    b_negdv = persist.tile([128, 1], F32)
    nc.vector.memset(b_negdv[:], -DELTA_V)

    dist_sb = persist.tile([128, cols], F32)
    h_bf = persist.tile([128, cols], BF16)
    S_sb = persist.tile([C, C], F32)
    means_all = persist.tile([C, E], F32)
    means_bf = persist.tile([C, E], BF16)
    means_fg4 = persist.tile([128, 32], BF16)
    Hs = persist.tile([4, 4 * C], F32)

    # ---- pass 1: segment sums + counts (bf16, accumulates S^T) --------
    G1 = G3
    n1 = cols // G1
    total1 = cols
    idx = 0
    with tc.tile_pool(name="p1", bufs=2) as p1p, tc.tile_pool(
        name="p1ps", bufs=1, space="PSUM"
    ) as pp1:
        psum_sT = pp1.tile([C, C], F32)
        for t in range(n1):
            xtf = p1p.tile([128, E, G1], F32, tag="xtf")
            nc.sync.dma_start(out=xtf[:], in_=emb_strided(t * G1, G1))
            xtb = p1p.tile([128, G1, C], BF16, tag="xtb")
            # cast f32 -> bf16, transposing (e, g) -> (g, e) via strided read
            nc.scalar.copy(
                out=_squeeze(xtb[:, :, 0:E]),
                in_=bass.AP(
                    tensor=xtf[:].tensor,
                    offset=xtf[:].offset,
                    ap=[xtf[:].ap[0], [1, G1], [G1, E]],
                ),
            )
            nc.gpsimd.memset(xtb[:, :, E : E + 1], 1.0)
            o1 = p1p.tile([128, C, G1], BF16, tag="o1")
            m_sl = m_bf[:, t * G1 : (t + 1) * G1]
            nc.vector.tensor_tensor(
                out=o1[:], in0=_ins(m_sl, [[0, C]]), in1=i33g[:], op=AOP.is_equal
            )
            for g in range(G1):
                nc.tensor.matmul(
                    psum_sT[:],
                    lhsT=_squeeze(xtb[:, g : g + 1, :]),
                    rhs=_squeeze(o1[:, :, g : g + 1]),
                    start=(idx == 0),
                    stop=(idx == total1 - 1),
                )
                idx += 1
        ST_sb = persist.tile([C, C], F32)
        nc.vector.tensor_copy(out=ST_sb[:], in_=psum_sT[:])
        psum_tr = pp1.tile([C, C], F32)
        nc.tensor.transpose(psum_tr[:], ST_sb[:], ident33[:])
        nc.vector.tensor_copy(out=S_sb[:], in_=psum_tr[:])
    nc.sync.dma_start(out=out_s, in_=S_sb[:])

    # ---- means --------------------------------------------------------
    cnt_m = persist.tile([C, 1], F32)
    nc.vector.tensor_scalar_max(cnt_m[:], S_sb[:, E : E + 1], 1.0)
    recip = persist.tile([C, 1], F32)
    nc.vector.reciprocal(out=recip[:], in_=cnt_m[:])
    nc.vector.tensor_scalar_mul(means_all[:], S_sb[:, 0:E], recip[:, 0:1])
    nc.vector.tensor_copy(out=means_bf[:], in_=means_all[:])
    # foreground means (instance r+1 -> row r), replicated to all 4 quadrants
    means_dram = dram_pool.tile([32, 32], BF16)
    nc.sync.dma_start(out=means_dram[:], in_=means_bf[1:C, :])
    for q in range(4):
        nc.sync.dma_start(
            out=means_fg4[32 * q : 32 * q + 32, :], in_=means_dram[:]
        )

    # ---- pass 2: per-pixel distances ----------------------------------
    hw2 = cols // 2
    nv = (hw2 + 511) // 512
    with tc.tile_pool(name="p2", bufs=2) as p2p, tc.tile_pool(
        name="p2mu", bufs=2, space="PSUM"
    ) as pmu, tc.tile_pool(name="p2d2", bufs=2, space="PSUM") as pd2:
        for s in range(32):
            x8 = p2p.tile([128, cols], F32, tag="x8")
            mbc = p2p.tile([128, cols], BF16, tag="mbc")
            for q in range(4):
                row = 4 * s + q
                nc.sync.dma_start(
                    out=x8[32 * q : 32 * q + 32, :],
                    in_=emb[:, row * cols : (row + 1) * cols],
                )
                src = m_dram[row : row + 1, :]
                nc.gpsimd.dma_start(
                    out=mbc[32 * q : 32 * q + 32, :],
                    in_=bass.AP(
                        tensor=src.tensor,
                        offset=src.offset,
                        ap=[[0, 32]] + src.ap[1:],
                    ),
                )
            o2 = p2p.tile([128, cols], BF16, tag="o2")
            nc.vector.tensor_tensor(
                out=o2[:],
                in0=mbc[:],
                in1=_ins(iota_r[:], [[0, cols // w_r]]),
                op=AOP.is_equal,
            )
            for h2 in range(2):
                muh = pmu.tile([128, hw2], F32, tag="mu")
                c0 = h2 * hw2
                for q in range(4):
                    for v in range(nv):
                        v0, v1 = v * 512, min((v + 1) * 512, hw2)
                        nc.tensor.matmul(
                            muh[32 * q : 32 * q + 32, v0:v1],
                            lhsT=means_fg4[32 * q : 32 * q + 32, :],
                            rhs=o2[32 * q : 32 * q + 32, c0 + v0 : c0 + v1],
                            start=True,
                            stop=True,
                            tile_position=(32 * q, 32 * q),
                        )
                diffh = p2p.tile([128, hw2], F32, tag="diff")
                nc.vector.tensor_tensor(
                    out=diffh[:], in0=x8[:, c0 : c0 + hw2], in1=muh[:], op=AOP.subtract
                )
                sqh = p2p.tile([128, hw2], F32, tag="sq")
                nc.scalar.activation(out=sqh[:], in_=diffh[:], func=ACT.Square)
                d2t = pd2.tile([4, hw2], F32, tag="d2")
                for v in range(nv):
                    v0, v1 = v * 512, min((v + 1) * 512, hw2)
                    nc.tensor.matmul(
                        d2t[:, v0:v1],
                        lhsT=blockones[:],
                        rhs=sqh[:, v0:v1],
                        start=True,
                        stop=True,
                    )
                dist_t = p2p.tile([4, hw2], F32, tag="dst")
                nc.scalar.activation(
                    out=dist_t[:], in_=d2t[:], func=ACT.Sqrt, bias=b_eps[:]
                )
                nc.sync.dma_start(
                    out=dist_sb[4 * s : 4 * s + 4, c0 : c0 + hw2], in_=dist_t[:]
                )
    r_t = persist.tile([128, cols], F32)
    nc.scalar.activation(
        out=r_t[:], in_=dist_sb[:], func=ACT.Relu, bias=b_negdv[:]
    )
    nc.vector.tensor_tensor(out=h_bf[:], in0=r_t[:], in1=r_t[:], op=AOP.mult)

    # ---- pass 3: per-instance hinge sums ------------------------------
    n3 = cols // G3
    nq = G3 // 4
    total3 = n3 * nq
    idx = 0
    with tc.tile_pool(name="p3", bufs=2) as p3p, tc.tile_pool(
        name="p3ps", bufs=1, space="PSUM"
    ) as pp3:
        psum_h = pp3.tile([4, 4 * C], F32)
        for t in range(n3):
            o3 = p3p.tile([128, C, G3], BF16, tag="o3")
            m_sl = m_bf[:, t * G3 : (t + 1) * G3]
            nc.vector.tensor_tensor(
                out=o3[:],
                in0=_ins(m_sl, [[0, C]]),
                in1=i33g[:],
                op=AOP.is_equal,
            )
            for i in range(nq):
                g0 = t * G3 + 4 * i
                nc.tensor.matmul(
                    psum_h[:],
                    lhsT=h_bf[:, g0 : g0 + 4],
                    rhs=o3[:, :, 4 * i : 4 * i + 4].rearrange("p c g -> p g c"),
                    start=(idx == 0),
                    stop=(idx == total3 - 1),
                )
                idx += 1
        nc.vector.tensor_copy(out=Hs[:], in_=psum_h[:])
    nc.sync.dma_start(out=out_h, in_=Hs[:])
    ctx.close()


def build_program(npix=N_PIX, loop_k=None):
    nc = bacc.Bacc("TRN2", debug=False, num_devices=8)
    emb = nc.dram_tensor("emb", (E, npix), F32, kind="ExternalInput").ap()
    mask = nc.dram_tensor("mask", (npix,), I32, kind="ExternalInput").ap()
    out_s = nc.dram_tensor("out_s", (C, C), F32, kind="ExternalOutput").ap()
    out_h = nc.dram_tensor("out_h", (4, 4 * C), F32, kind="ExternalOutput").ap()
    with tile.TileContext(nc) as tc:
        if loop_k is None:
            build_body(tc, nc, emb, mask, out_s, out_h, npix)
        else:
            with tc.For_i(0, loop_k, 1):
                build_body(tc, nc, emb, mask, out_s, out_h, npix)
    nc.compile()
    return nc


_program = None


def _get_program():
    global _program
    if _program is None:
        _program = build_program()
    return _program


def finalize_host(S, Hdiag):
    """Per-sample host math.  S: (C, C) [sums | counts], Hdiag: (4, 4*C)."""
    S = S.astype(np.float64)
    Hh = Hdiag.astype(np.float64)
    Hc = np.zeros(C)
    for i in range(4):
        Hc += Hh[i, C * i : C * i + C]
    counts = S[:, E]
    sums = S[:, 0:E]
    means_all = sums / np.maximum(counts, 1.0)[:, None]
    means = means_all[1:]
    cnt = counts[1:]
    present = cnt > 0

    per_inst = Hc[1:] / np.maximum(cnt, 1.0)
    n_present = max(present.sum(), 1.0)
    var_loss = np.where(present, per_inst, 0.0).sum() / n_present

    d = means[:, None, :] - means[None, :, :]
    sq = (d * d).sum(-1)
    iu = np.triu(np.ones((NI, NI), bool), k=1)
    pd = np.sqrt(np.where(iu, np.maximum(sq, 1e-12), 1.0))
    hinged_d = np.where(iu, np.maximum(2.0 * DELTA_D - pd, 0.0) ** 2, 0.0)
    dist_loss = hinged_d.sum() / (NI * (NI - 1) / 2.0)

    reg_loss = np.sqrt((means * means).sum(1) + 1e-12).mean()
    return var_loss, dist_loss, reg_loss


_last_results = None


def kernel(embeddings, instance_masks, trace=False):
    global _last_results
    nc = _get_program()
    embeddings = np.ascontiguousarray(np.asarray(embeddings), dtype=np.float32)
    instance_masks = np.ascontiguousarray(
        np.asarray(instance_masks), dtype=np.int32
    )
    in_maps = [
        {
            "emb": embeddings[b].reshape(E, N_PIX),
            "mask": instance_masks[b].reshape(N_PIX),
        }
        for b in range(B)
    ]
    res = run_bass_kernel_spmd(nc, in_maps, core_ids=list(range(B)), trace=trace)
    _last_results = res
    var = np.zeros(B)
    dst = np.zeros(B)
    reg = np.zeros(B)
    for b in range(B):
        var[b], dst[b], reg[b] = finalize_host(
            res.results[b]["out_s"], res.results[b]["out_h"]
        )
    var_loss = var.mean()
    dist_loss = dst.mean()
    reg_loss = reg.mean()
    total = ALPHA * var_loss + BETA * dist_loss + GAMMA * reg_loss
    return (
        np.float32(total),
        np.float32(var_loss),
        np.float32(dist_loss),
        np.float32(reg_loss),
    )
